# revision 3
# baseline (speedup 1.0000x reference)
"""Adaptive embedding lookup on 8 TRN2 NeuronCores.

Strategy (data-parallel over tokens, tables replicated):
  - input_ids is [8, 4096]; core k handles batch row k (4096 tokens).
  - On host, each core's tokens are partitioned by cluster:
      cluster 0: id in [0, 20000)       -> emb0 row, copied through
      cluster 1: id in [20000, 40000)   -> emb1 row @ proj1.T
      cluster 2: id in [40000, 50000)   -> emb2 row @ proj2.T
    Padding-idx tokens (local row 1 of each table) are routed to an
    appended all-zero table row, so every token's output row is written
    explicitly by the device.
  - On device, per 128-token tile: indirect-DMA gather of table rows,
    (clusters 1/2) PE transpose + bf16 matmul against the projection,
    then indirect-DMA scatter of the finished [128, 1024] rows into the
    per-core output at the tokens' positions.
  - SPMD: all 8 cores run one graph; per-cluster tile counts are padded
    to the max across cores. Padded gather lanes read the zero row and
    padded scatter lanes write an extra trash row (row 4096) of the
    per-core output, which is dropped on the host.
"""

import os

import numpy as np

N_CORES = 8
B, S = 8, 4096
CUT0, CUT1, VOCAB = 20000, 40000, 50000
D = 1024
D1, D2 = 256, 64
PAD = 1

Z0, Z1, Z2 = 20000, 20000, 10000  # appended zero-row index per table
TRASH_ROW = S  # scatter target for padded lanes

LAST_EXEC_NS = None
LAST_RESULT = None


def _pack_lanes(vals: np.ndarray, n_tiles: int, fill: int) -> np.ndarray:
    """Pad to n_tiles*128 and lay out as [128, n_tiles] (tile j = col j)."""
    out = np.full(n_tiles * 128, fill, dtype=np.int32)
    out[: len(vals)] = vals.astype(np.int32)
    return np.ascontiguousarray(out.reshape(n_tiles, 128).T)


def _build(nc, n0: int, n1: int, n2: int):
    from concourse import mybir, tile
    from concourse.bass import IndirectOffsetOnAxis
    from concourse.masks import make_identity

    f32 = mybir.dt.float32
    bf16 = mybir.dt.bfloat16
    i32 = mybir.dt.int32

    emb0p = nc.dram_tensor("emb0p", [Z0 + 1, D], f32, kind="ExternalInput")
    emb1p = nc.dram_tensor("emb1p", [Z1 + 1, D1], f32, kind="ExternalInput")
    emb2p = nc.dram_tensor("emb2p", [Z2 + 1, D2], f32, kind="ExternalInput")
    p1t = nc.dram_tensor("p1t", [D1, D], f32, kind="ExternalInput")
    p2t = nc.dram_tensor("p2t", [D2, D], f32, kind="ExternalInput")
    idx0 = nc.dram_tensor("idx0", [128, n0], i32, kind="ExternalInput")
    pos0 = nc.dram_tensor("pos0", [128, n0], i32, kind="ExternalInput")
    idx1 = nc.dram_tensor("idx1", [128, n1], i32, kind="ExternalInput")
    pos1 = nc.dram_tensor("pos1", [128, n1], i32, kind="ExternalInput")
    idx2 = nc.dram_tensor("idx2", [128, n2], i32, kind="ExternalInput")
    pos2 = nc.dram_tensor("pos2", [128, n2], i32, kind="ExternalInput")
    out_e = nc.dram_tensor("out", [S + 1, D], f32, kind="ExternalOutput")

    with tile.TileContext(nc) as tc:
        with (
            tc.tile_pool(name="const", bufs=1) as cpool,
            tc.tile_pool(name="g0", bufs=3) as g0pool,
            tc.tile_pool(name="g12", bufs=3) as g12pool,
            tc.tile_pool(name="eT", bufs=3) as eTpool,
            tc.tile_pool(name="o", bufs=4) as opool,
            tc.tile_pool(name="pt", bufs=2, space="PSUM") as ptpool,
            tc.tile_pool(name="po", bufs=4, space="PSUM") as popool,
        ):
            idx0_sb = cpool.tile([128, n0], i32)
            pos0_sb = cpool.tile([128, n0], i32)
            idx1_sb = cpool.tile([128, n1], i32)
            pos1_sb = cpool.tile([128, n1], i32)
            idx2_sb = cpool.tile([128, n2], i32)
            pos2_sb = cpool.tile([128, n2], i32)
            nc.sync.dma_start(out=idx0_sb[:], in_=idx0[:])
            nc.sync.dma_start(out=pos0_sb[:], in_=pos0[:])
            nc.sync.dma_start(out=idx1_sb[:], in_=idx1[:])
            nc.sync.dma_start(out=pos1_sb[:], in_=pos1[:])
            nc.sync.dma_start(out=idx2_sb[:], in_=idx2[:])
            nc.sync.dma_start(out=pos2_sb[:], in_=pos2[:])

            ident = cpool.tile([128, 128], f32)
            make_identity(nc, ident[:])

            p1f = cpool.tile([128, 2, D], f32)
            nc.sync.dma_start(out=p1f[:, 0, :], in_=p1t[0:128, :])
            nc.sync.dma_start(out=p1f[:, 1, :], in_=p1t[128:256, :])
            p1b = cpool.tile([128, 2, D], bf16)
            nc.vector.tensor_copy(out=p1b[:], in_=p1f[:])

            p2f = cpool.tile([D2, D], f32)
            nc.sync.dma_start(out=p2f[:], in_=p2t[:])
            p2b = cpool.tile([D2, D], bf16)
            nc.vector.tensor_copy(out=p2b[:], in_=p2f[:])

            # ---- cluster 0: gather emb0 rows, scatter straight out ----
            # NOTE: indirect DMA offsets must be [128, 1] (one row per
            # partition); multi-column offset APs mis-gather on HW.
            for j in range(n0):
                g0 = g0pool.tile([128, D], f32, tag="g0")
                nc.gpsimd.indirect_dma_start(
                    out=g0[:],
                    out_offset=None,
                    in_=emb0p[:],
                    in_offset=IndirectOffsetOnAxis(ap=idx0_sb[:, j : j + 1], axis=0),
                )
                nc.gpsimd.indirect_dma_start(
                    out=out_e[:],
                    out_offset=IndirectOffsetOnAxis(ap=pos0_sb[:, j : j + 1], axis=0),
                    in_=g0[:],
                    in_offset=None,
                )

            # ---- cluster 1: gather emb1, transpose, matmul proj1, scatter ----
            for j in range(n1):
                g1 = g12pool.tile([128, D1], f32, tag="g1")
                nc.gpsimd.indirect_dma_start(
                    out=g1[:],
                    out_offset=None,
                    in_=emb1p[:],
                    in_offset=IndirectOffsetOnAxis(ap=idx1_sb[:, j : j + 1], axis=0),
                )
                eT = eTpool.tile([128, 2, 128], bf16, tag="eT")
                for c in range(2):
                    tp = ptpool.tile([128, 128], f32, tag="tp")
                    nc.tensor.transpose(
                        out=tp[:], in_=g1[:, c * 128 : (c + 1) * 128], identity=ident[:]
                    )
                    nc.vector.tensor_copy(out=eT[:, c, :], in_=tp[:])
                o1 = opool.tile([128, D], f32, tag="o")
                for nn in range(2):
                    om = popool.tile([128, 512], f32, tag="om")
                    nc.tensor.matmul(
                        out=om[:],
                        lhsT=eT[:, 0, :],
                        rhs=p1b[:, 0, nn * 512 : (nn + 1) * 512],
                        start=True,
                        stop=False,
                    )
                    nc.tensor.matmul(
                        out=om[:],
                        lhsT=eT[:, 1, :],
                        rhs=p1b[:, 1, nn * 512 : (nn + 1) * 512],
                        start=False,
                        stop=True,
                    )
                    nc.scalar.copy(out=o1[:, nn * 512 : (nn + 1) * 512], in_=om[:])
                nc.gpsimd.indirect_dma_start(
                    out=out_e[:],
                    out_offset=IndirectOffsetOnAxis(ap=pos1_sb[:, j : j + 1], axis=0),
                    in_=o1[:],
                    in_offset=None,
                )

            # ---- cluster 2: gather emb2, transpose, matmul proj2, scatter ----
            for j in range(n2):
                g2 = g12pool.tile([128, D2], f32, tag="g2")
                nc.gpsimd.indirect_dma_start(
                    out=g2[:],
                    out_offset=None,
                    in_=emb2p[:],
                    in_offset=IndirectOffsetOnAxis(ap=idx2_sb[:, j : j + 1], axis=0),
                )
                eT2 = eTpool.tile([D2, 128], bf16, tag="eT2")
                tp2 = ptpool.tile([D2, 128], f32, tag="tp2")
                nc.tensor.transpose(out=tp2[:], in_=g2[:], identity=ident[:])
                nc.vector.tensor_copy(out=eT2[:], in_=tp2[:])
                o2 = opool.tile([128, D], f32, tag="o")
                for nn in range(2):
                    om = popool.tile([128, 512], f32, tag="om")
                    nc.tensor.matmul(
                        out=om[:],
                        lhsT=eT2[:],
                        rhs=p2b[:, nn * 512 : (nn + 1) * 512],
                        start=True,
                        stop=True,
                    )
                    nc.scalar.copy(out=o2[:, nn * 512 : (nn + 1) * 512], in_=om[:])
                nc.gpsimd.indirect_dma_start(
                    out=out_e[:],
                    out_offset=IndirectOffsetOnAxis(ap=pos2_sb[:, j : j + 1], axis=0),
                    in_=o2[:],
                    in_offset=None,
                )

    return out_e


def _prep_core(ids_k: np.ndarray):
    """Partition one core's token ids into per-cluster (local_row, position)."""
    m0 = ids_k < CUT0
    m1 = (ids_k >= CUT0) & (ids_k < CUT1)
    m2 = ids_k >= CUT1
    res = []
    for m, base, zrow in ((m0, 0, Z0), (m1, CUT0, Z1), (m2, CUT1, Z2)):
        pos = np.nonzero(m)[0].astype(np.int32)
        loc = (ids_k[m].astype(np.int64) - base).astype(np.int32)
        loc[loc == PAD] = zrow  # padding_idx rows must come out zero
        res.append((loc, pos))
    return res


def kernel(input_ids, emb0, emb1, emb2, proj1, proj2):
    global LAST_EXEC_NS, LAST_RESULT
    from concourse import bacc
    from concourse.bass_utils import run_bass_kernel_spmd

    input_ids = np.asarray(input_ids)
    assert input_ids.shape == (B, S), input_ids.shape

    emb0p = np.concatenate([emb0, np.zeros((1, D), np.float32)], axis=0)
    emb1p = np.concatenate([emb1, np.zeros((1, D1), np.float32)], axis=0)
    emb2p = np.concatenate([emb2, np.zeros((1, D2), np.float32)], axis=0)
    p1t = np.ascontiguousarray(proj1.T.astype(np.float32))
    p2t = np.ascontiguousarray(proj2.T.astype(np.float32))

    preps = [_prep_core(input_ids[k]) for k in range(N_CORES)]
    n_tiles = [
        max(1, -(-max(len(preps[k][c][0]) for k in range(N_CORES)) // 128))
        for c in range(3)
    ]
    n0, n1, n2 = n_tiles

    in_maps = []
    for k in range(N_CORES):
        (l0, q0), (l1, q1), (l2, q2) = preps[k]
        in_maps.append(
            {
                "emb0p": emb0p,
                "emb1p": emb1p,
                "emb2p": emb2p,
                "p1t": p1t,
                "p2t": p2t,
                "idx0": _pack_lanes(l0, n0, Z0),
                "pos0": _pack_lanes(q0, n0, TRASH_ROW),
                "idx1": _pack_lanes(l1, n1, Z1),
                "pos1": _pack_lanes(q1, n1, TRASH_ROW),
                "idx2": _pack_lanes(l2, n2, Z2),
                "pos2": _pack_lanes(q2, n2, TRASH_ROW),
            }
        )

    nc = bacc.Bacc("TRN2", target_bir_lowering=False, debug=False, num_devices=N_CORES)
    _build(nc, n0, n1, n2)
    nc.compile()

    trace = bool(os.environ.get("EMB_KERNEL_TRACE"))
    res = run_bass_kernel_spmd(nc, in_maps, list(range(N_CORES)), trace=trace)
    LAST_RESULT = res
    LAST_EXEC_NS = res.exec_time_ns

    out = np.stack([res.results[k]["out"][:S] for k in range(N_CORES)], axis=0)
    return out
